# revision 27
# baseline (speedup 1.0000x reference)
"""Trainium2 Bass kernel for ChebConv with spatial attention.

Reference computation (per sample b):
    A_k = cheb[k] * att[b]                    (elementwise, [N,N])
    rhs_k = A_k @ x[b,t]                      ([N,N] @ [N,F_IN] for all t)
    out[b,t] = relu(sum_k rhs_k @ Theta[k])   ([N,F_OUT])

Sharding: data-parallel over batch B=8, one sample per NeuronCore.
Host prep computes the attention-weighted adjacencies A_k = cheb_k*att
in fp32 and ships their transposes atT (layout [j,i], bf16) directly,
so the chip does no elementwise products at all; the PE consumes atT
tiles as the moving matmul operand with contraction over j on the
partition dim. All matmuls run in bf16 (single-pass on the PE, fp32
PSUM accumulation); the relu'd output is stored bf16 on-chip and
upcast to fp32 on the host.

Per-core dataflow:
  phase B: for k, j-group: accumulate RT[(t,f)=128, i=512] =
           X_tile^T @ atT_k into PSUM over j-tiles (N=512 bf16
           matmuls), then copy+cast PSUM->SBUF bf16 (DVE + ACT; the
           last k's copies are column-chunked so phase C can start on
           early i-blocks while later columns still copy).
  phase C: out[i=128, (t,o)] += RT^T @ thetaM_k, where thetaM zero-pads
           Theta[k] per 32-row strip so one full-K matmul produces the
           4 t's of a t-group (N=256). relu split DVE/ACT, contiguous
           DMA out.

DMA layout: atT/x loaded as grouped tiles covering 1-2 128-row j-tiles
per transfer (two narrow leading groups so the first matmul starts
after ~0.4 MB; rearranged so each partition carries all rows of its
group); transfers alternate between the sync and scalar HWDGE queues
so issue overhead and queue bandwidth are split. No explicit PE
warm-up: the PE queue is blocked by the framework preamble until
~7.8us anyway, so the first real j-tile's matmuls (which land right
around then) do the HAM warm-up themselves.
"""

import numpy as np
from contextlib import ExitStack

B, T, N, F_IN, F_OUT, K = 8, 16, 1024, 32, 64, 3
NJ = N // 128  # j tiles (contraction)
NI = N // 128  # i tiles (output rows)
NTG = 4        # t-groups of 4 t's -> 128 = 4*32 partitions
TF = T * F_IN   # 512
TO = T * F_OUT  # 1024
W = 2           # j-tiles per wide DMA

_LAST_RESULTS = None  # BassKernelResults of the most recent run (for test harness)


def _build_bass():
    import concourse.mybir as mybir
    import concourse.tile as tile
    from concourse import bacc
    from concourse.bass import ts

    f32 = mybir.dt.float32
    bf16 = mybir.dt.bfloat16
    nc = bacc.Bacc()

    xT_d = nc.dram_tensor("xT", [N, TF], bf16, kind="ExternalInput")
    atT_d = nc.dram_tensor("atT", [K * N, N], bf16, kind="ExternalInput")
    th_d = nc.dram_tensor("thetaM", [128, K * 4 * F_OUT], bf16, kind="ExternalInput")
    out_d = nc.dram_tensor("out", [N, TO], bf16, kind="ExternalOutput")

    # j-tiles grouped per DMA: four narrow leading groups smooth the k=0
    # feed-demand curve (the head is DMA-latency-bound); the rest go wide
    GROUPS = [(0,), (1,), (2,), (3,), (4, 5), (6, 7)]

    def grouped(dram, row0, L):  # L j-tiles -> [128, L, cols]
        return dram[row0:row0 + L * 128, :].rearrange("(a p) n -> p a n", p=128)

    def g3(t, L):  # view a grouped SBUF tile as [128, L, cols]
        return t[:].rearrange("p (a n) -> p a n", a=L)

    with tile.TileContext(nc) as tc, ExitStack() as ctx:
        x_pool = ctx.enter_context(tc.tile_pool(name="x", bufs=1))
        at_pool = ctx.enter_context(tc.tile_pool(name="at", bufs=6))
        rt_pool = ctx.enter_context(tc.tile_pool(name="rt", bufs=K * NTG))
        th_pool = ctx.enter_context(tc.tile_pool(name="th", bufs=1))
        ob_pool = ctx.enter_context(tc.tile_pool(name="ob", bufs=3))
        wz_pool = ctx.enter_context(tc.tile_pool(name="wz", bufs=1))

        xg = [None] * len(GROUPS)

        # phase B: RT[k][tg] = X[:, tg-block]^T @ atT_k
        rts = [[None] * NTG for _ in range(K)]
        th = None
        if True:
            pb = ctx.enter_context(
                tc.tile_pool(name="psumB", bufs=1, space="PSUM"))
            # PE warm-up: ~3us of zero matmuls during the unavoidable DMA
            # head (start barrier ~6.3us + first-tile transfer+receipt
            # ~3us) so HAM un-throttles (1.2 -> 2.4 GHz) before the first
            # real matmul. Shares chain7's PSUM slot; that chain's first
            # real write comes ~1.7us after the warm-up ends.
            wz = wz_pool.tile([128, 512], bf16, name="warmz")
            nc.gpsimd.memset(wz[:], 0)
            wps = pb.tile([128, 512], f32, name="warmps", tag="chain7")
            for _ in range(6):
                nc.tensor.matmul(wps[:], wz[:, 0:128], wz[:], start=True, stop=True)
            # SDMA engines round-robin EQUALLY across active queue rows,
            # so spreading the k=0 head over several queues just dilutes
            # each one. The whole k=0 critical sequence rides the sync
            # queue alone in strict need-order; k=1 goes to scalar and
            # k=2 back to sync. Each next-k DMA is emitted BEFORE the
            # current k's PSUM copies (no head-of-line blocking behind
            # them on the scalar queue), and the at-pool ring (bufs = one
            # k's worth of tiles) self-throttles transfer (k+1,g) to
            # start only once (k,g)'s matmuls have read its buffer.
            at_tiles = {}

            def issue_at(ka, ga):
                grp_a = GROUPS[ga]
                La = len(grp_a)
                at = at_pool.tile([128, La * N], bf16, name=f"at{ka}_{ga}",
                                  tag="at", padded_shape=[128, W * N])
                if ka == 0 and ga == 0:
                    # split the very first adjacency tile into two
                    # half-column DMAs so the ih=0 matmuls can start
                    # as soon as the first 128 KB lands
                    nc.sync.dma_start(at[:, 0:512], atT_d[0:128, 0:512])
                    nc.sync.dma_start(at[:, 512:1024], atT_d[0:128, 512:1024])
                else:
                    q = nc.scalar if ka == 1 else nc.sync
                    q.dma_start(g3(at, La),
                                grouped(atT_d, ka * N + grp_a[0] * 128, La))
                at_tiles[(ka, ga)] = at

            for k in range(K):
                chains = [
                    pb.tile([128, 512], f32, name=f"chain{k}_{c}", tag=f"chain{c}")
                    for c in range(2 * NTG)
                ]
                for g, grp in enumerate(GROUPS):
                    L = len(grp)
                    if k == 0:
                        xt = x_pool.tile([128, L * TF], bf16,
                                         name=f"xg{g}", tag=f"xg{g}")
                        nc.sync.dma_start(g3(xt, L),
                                          grouped(xT_d, grp[0] * 128, L))
                        xg[g] = xt
                        issue_at(0, g)
                        if g == len(GROUPS) - 1:
                            th = th_pool.tile([128, K * 4 * F_OUT], bf16)
                            nc.sync.dma_start(th[:], th_d[:, :])
                    at = at_tiles.pop((k, g))
                    for js in range(L):
                        first = g == 0
                        last = (g == len(GROUPS) - 1 and js == L - 1)
                        if first:
                            # ih-major: the first 4 matmuls only need the
                            # first half-column DMA of at00
                            order = [(tg, ih) for ih in range(2)
                                     for tg in range(NTG)]
                        elif last and k == K - 1:
                            # stop chains in the order the PSUM->SBUF
                            # copies (and phase C's first i-blocks) want
                            # them: c7/c1 free the banks psA{0}/psB{0}
                            # reuse, then the ih=0 chains phase C reads
                            order = [(3, 1), (0, 1), (0, 0), (1, 0),
                                     (2, 0), (3, 0), (1, 1), (2, 1)]
                        else:
                            order = [(tg, ih) for tg in range(NTG)
                                     for ih in range(2)]
                        for tg, ih in order:
                            nc.tensor.matmul(
                                chains[tg * 2 + ih][:],
                                xg[g][:, ts(js * NTG + tg, 128)],
                                at[:, ts(js * 2 + ih, 512)],
                                start=first,
                                stop=last,
                            )
                if k + 1 < K:
                    # emit next k's at-DMAs BEFORE this k's copies
                    for ga in range(len(GROUPS)):
                        issue_at(k + 1, ga)
                for tg in range(NTG):
                    rt = rt_pool.tile([128, N], bf16)
                    if k < K - 1:
                        nc.vector.tensor_copy(rt[:, 0:512], chains[tg * 2][:])
                        nc.scalar.copy(rt[:, 512:1024], chains[tg * 2 + 1][:])
                    rts[k][tg] = rt
                if k == K - 1:
                    # column-chunked copies, sequenced per engine so that
                    # (1) chains c7/c1 — whose PSUM banks phase C's
                    # psA{0}/psB{0} reuse — are fully read first, and
                    # (2) the ih=0 column-0 chunks phase C's first
                    # i-blocks read come right after. Chain stop order in
                    # the last j-group above matches this sequence.
                    DVE_SEQ = [(7, 0), (7, 256), (0, 0), (4, 0),
                               (0, 256), (4, 256), (5, 0), (5, 256)]
                    ACT_SEQ = [(1, 0), (1, 256), (2, 0), (6, 0),
                               (2, 256), (6, 256), (3, 0), (3, 256)]
                    for (cd, lod), (ca, loa) in zip(DVE_SEQ, ACT_SEQ):
                        for c, lo, eng in ((cd, lod, 'v'), (ca, loa, 's')):
                            tg, ih = c // 2, c % 2
                            dst = rts[k][tg][:, ih * 512 + lo:ih * 512 + lo + 256]
                            src = chains[c][:, lo:lo + 256]
                            if eng == 'v':
                                nc.vector.tensor_copy(dst, src)
                            else:
                                nc.scalar.copy(dst, src)

        # phase C: out[i-block, (t,o)] = relu(sum_k RT_k^T @ thetaM_k).
        # One matmul per (tg, k): full K=128 contraction where thetaM
        # zero-pads Theta[k] per 32-row strip, producing the 4 t's of
        # the t-group in one N=256 matmul. Two single-bank PSUM tiles
        # per i-block; tg order alternates banks so only one
        # accumulation group is open per bank.
        if True:
            # phase C psum tiles reuse phase B's chain banks via explicit
            # tags, pinned so the first i-blocks take the banks whose k2
            # copies complete first (c7/c1 lead both engines' sequences);
            # even/odd ib alternate banks for double buffering
            for ib in range(NI):
                psA = pb.tile([128, 512], f32, name=f"psA{ib}",
                              tag="chain7" if ib % 2 == 0 else "chain0")
                psB = pb.tile([128, 512], f32, name=f"psB{ib}",
                              tag="chain1" if ib % 2 == 0 else "chain2")
                for tg, ps in ((0, psA), (2, psB), (1, psA), (3, psB)):
                    for k in range(K):
                        nc.tensor.matmul(
                            ps[:, ts(tg % 2, 4 * F_OUT)],
                            rts[k][tg][:, ts(ib, 128)],
                            th[:, ts(k, 4 * F_OUT)],
                            start=(k == 0),
                            stop=(k == K - 1),
                        )
                ob = ob_pool.tile([128, TO], bf16)
                nc.vector.tensor_relu(ob[:, 0:512], psA[:])
                nc.scalar.activation(ob[:, 512:1024], psB[:],
                                     mybir.ActivationFunctionType.Relu)
                if ib < NI - 1:
                    nc.sync.dma_start(out_d[ts(ib, 128), :], ob[:])
                else:
                    # last block: DMA the psA half (finishes first) on
                    # sync while the psB half relu is still running, so
                    # the final transfer's ~2us receipt starts earlier
                    nc.sync.dma_start(out_d[ts(ib, 128), 0:512],
                                      ob[:, 0:512])
                    nc.scalar.dma_start(out_d[ts(ib, 128), 512:1024],
                                        ob[:, 512:1024])

    nc.compile()
    return nc


def _prep_inputs(x, att, cheb, Theta):
    import ml_dtypes

    bf16 = ml_dtypes.bfloat16
    # zero-padded Theta: strip tt of the partition dim carries Theta[k]
    # only in the tt-th 64-col block of k's 256-col group
    thetaM = np.zeros((128, K * 4 * F_OUT), np.float32)
    for tt in range(4):
        for k in range(K):
            thetaM[tt * 32:(tt + 1) * 32,
                   k * 4 * F_OUT + tt * F_OUT:
                   k * 4 * F_OUT + (tt + 1) * F_OUT] = Theta[k]
    thetaM = thetaM.astype(bf16)

    in_maps = []
    for b in range(B):
        # attention-weighted adjacencies, transposed to [j, i], fp32
        # product rounded once to bf16
        atT = (cheb * att[b][None]).transpose(0, 2, 1)
        atT = np.ascontiguousarray(atT).reshape(K * N, N).astype(bf16)
        in_maps.append({
            "xT": np.ascontiguousarray(
                x[b].transpose(1, 0, 2)).reshape(N, TF).astype(bf16),
            "atT": atT,
            "thetaM": thetaM,
        })
    return in_maps


def kernel(**inputs: np.ndarray) -> np.ndarray:
    global _LAST_RESULTS
    from concourse.bass_utils import run_bass_kernel_spmd

    x = np.asarray(inputs["x"], dtype=np.float32)
    att = np.asarray(inputs["spatial_attention"], dtype=np.float32)
    cheb = np.asarray(inputs["cheb"], dtype=np.float32)
    Theta = np.asarray(inputs["Theta"], dtype=np.float32)

    in_maps = _prep_inputs(x, att, cheb, Theta)
    nc = _build_bass()
    res = run_bass_kernel_spmd(nc, in_maps, core_ids=list(range(B)))
    _LAST_RESULTS = res

    out = np.stack(
        [r["out"].astype(np.float32).reshape(N, T, F_OUT).transpose(1, 0, 2)
         for r in res.results]
    )
    return out
